# revision 32
# baseline (speedup 1.0000x reference)
"""Trainium2 Bass kernel: Conv3d(3->24, k=3, valid) + bias -> min over depth -> softmax over channels.

Full inputs: x (128, 3, 16, 64, 64) f32, conv_weight (24, 3, 3, 3, 3), conv_bias (24,).
Output: (128, 24, 62, 62) f32.

Data-parallel over 8 cores (16 batch each). Per core:
  Conv as TensorE matmul, K = 63 partitions = 3 depth-taps x (7 h-rows x 3 in-chans),
  M = 128 = (5 h-outputs x 24 out-chans + 8 pad), N = 496 = (8 batch x 62 w-outputs).
  kw handled by 3 PSUM-accumulating passes with shifted rhs offsets; depth (14 outputs) sequential,
  each step loading a fresh pool-buffered 3-plane tile (deep DMA prefetch; ~3x input re-read is
  still under the PE roofline and removes load latency from the critical path).
  Two row-tiled PE units (SBUF partitions 0-62 / 64-126) process the two batch octets concurrently;
  slot loads are per-unit 3-dim DMAs with 2KB contiguous (b, w) runs (host pre-transposes x to
  (d, h, c, b, w)), issued on the SP HWDGE ring (unit 0) and gpsimd SWDGE (unit 1).
  Epilogue: min over depth split across ScalarE (psum->sbuf fp16 copies) and VectorE (running min),
  exp with per-partition bias, block-diag ones-matmul for softmax denominators, fast reciprocal,
  multiply; output written in (h, o, b, w) layout and transposed back on host.
"""

import numpy as np

import concourse.bacc as bacc
import concourse.bass as bass
import concourse.mybir as mybir
import concourse.tile as tile
from concourse.bass_utils import run_bass_kernel_spmd

F32 = mybir.dt.float32
F32R = mybir.dt.float32r
FP16 = mybir.dt.float16
BF16 = mybir.dt.bfloat16

B_CORE = 16
C_IN = 3
D_IN = 16
H_IN = 64
W_IN = 64
O = 24
D_OUT = 14
HW_OUT = 62
HB = [0, 5, 10, 15, 20, 25, 30, 35, 40, 45, 50, 55, 57]
N_FREE = 8 * HW_OUT  # 496

_CACHE = {}


def build_host_tensors(conv_weight, conv_bias):
    """lhsT variants (kw) with partition order p = kd*21 + j*3 + c."""
    L = np.zeros((3, 63, 128), np.float32)
    for kw in range(3):
        for kd in range(3):
            for c in range(C_IN):
                for j in range(7):
                    for hp in range(5):
                        kh = j - hp
                        if 0 <= kh <= 2:
                            L[kw, kd * 21 + j * 3 + c, hp * 24:hp * 24 + O] = \
                                conv_weight[:, c, kd, kh, kw]
    ones = np.zeros((128, 128), np.float32)
    for hp in range(5):
        ones[hp * 24:(hp + 1) * 24, hp * 24:(hp + 1) * 24] = 1.0
    biasv = np.zeros((128, 1), np.float32)
    for hp in range(5):
        biasv[hp * 24:(hp + 1) * 24, 0] = conv_bias
    Lp = np.ascontiguousarray(L.transpose(1, 0, 2).reshape(63, 3 * 128))
    return Lp, ones, biasv


def build_bass():
    nc = bacc.Bacc(None, target_bir_lowering=False)
    # x pre-transposed on host to (d, h, c, b, w): one slot DMA covers both units
    # with 3-dim APs and 2KB contiguous (b, w) runs.
    x = nc.dram_tensor("x", [D_IN, H_IN, C_IN, B_CORE, W_IN], BF16, kind="ExternalInput")
    lw = nc.dram_tensor("lw", [63, 3 * 128], BF16, kind="ExternalInput")
    ones = nc.dram_tensor("ones", [128, 128], FP16, kind="ExternalInput")
    biasv = nc.dram_tensor("biasv", [128, 1], F32, kind="ExternalInput")
    y = nc.dram_tensor("y", [HW_OUT, O, B_CORE, HW_OUT], F32, kind="ExternalOutput")

    with tile.TileContext(nc) as tc:
        with (
            tc.tile_pool(name="const", bufs=1) as constp,
            tc.tile_pool(name="xs", bufs=4) as xsp,
            tc.tile_pool(name="mins", bufs=2) as minsp,
            tc.tile_pool(name="acp", bufs=2) as acp,
            tc.tile_pool(name="evt", bufs=4) as evp,
            tc.tile_pool(name="outp", bufs=4) as outp,
            tc.tile_pool(name="ps", bufs=3, space="PSUM") as psp,
            tc.tile_pool(name="psd", bufs=1, space="PSUM") as psdp,
        ):
            lwt = constp.tile([128, 3 * 128], BF16, tag="lw")
            onest = constp.tile([128, 128], FP16, tag="ones")
            biast = constp.tile([128, 1], F32, tag="bias")
            for base in (0, 64):
                nc.sync.dma_start(lwt[base:base + 63, :], lw[:, :])
            nc.sync.dma_start(onest[:, :], ones[:, :])
            nc.sync.dma_start(biast[:, :], biasv[:, :])

            for h0 in HB:
                accs = [None, None]
                stash = [[], []]
                for dt in range(D_OUT):
                    xt = xsp.tile([128, 512], BF16, tag="xt", name="xt")
                    for unit, eng in ((0, nc.sync), (1, nc.gpsimd)):
                        base = 64 * unit
                        srcap = x[dt:dt + 3, h0:h0 + 7, :,
                                  unit * 8:unit * 8 + 8, :].rearrange(
                            "p j c b w -> p (j c) (b w)")
                        eng.dma_start(xt[base:base + 63, :], srcap)
                    for unit in range(2):
                        base = 64 * unit
                        ps = psp.tile([128, N_FREE], F32, tag=f"ps{unit}",
                                      name=f"ps{unit}")
                        for kw in range(3):
                            lhsT = lwt[base:base + 63,
                                       kw * 128:(kw + 1) * 128]
                            rhs = xt[base:base + 63, :].rearrange(
                                "p (b w) -> p b w", b=8)[:, :, kw:kw + HW_OUT]
                            nc.tensor.matmul(
                                ps[:, :], lhsT, rhs,
                                start=(kw == 0), stop=(kw == 2),
                                tile_position=(base, 0))
                        # min-chain: even dt (and 13) -> ScalarE copy (fp16), odd -> VectorE min
                        if dt % 2 == 0 or dt == 13:
                            a = acp.tile([128, N_FREE], FP16, tag=f"a{unit}",
                                         name=f"a{unit}", bufs=16)
                            nc.scalar.copy(a[:, :], ps[:, :])
                            stash[unit].append(a)
                        else:
                            acc = accs[unit]
                            if acc is None:
                                acc = minsp.tile([128, N_FREE], FP16,
                                                 tag=f"mins{unit}", name=f"mins{unit}")
                                nc.vector.tensor_tensor(
                                    acc[:, :], ps[:, :], stash[unit].pop(0)[:, :],
                                    mybir.AluOpType.min)
                                accs[unit] = acc
                            else:
                                nc.vector.tensor_tensor(
                                    acc[:, :], ps[:, :], acc[:, :],
                                    mybir.AluOpType.min)
                for unit in range(2):
                    b0 = unit * 8
                    acc = accs[unit]
                    for a in stash[unit]:
                        nc.vector.tensor_tensor(
                            acc[:, :], a[:, :], acc[:, :], mybir.AluOpType.min)
                    et = evp.tile([128, N_FREE], FP16, tag=f"e{unit}", name=f"et{unit}")
                    nc.scalar.activation(et[:, :], acc[:, :],
                                         mybir.ActivationFunctionType.Exp,
                                         bias=biast[:, 0:1], scale=1.0)
                    dps = psdp.tile([128, N_FREE], F32, tag=f"dps{unit}",
                                    name=f"dps{unit}")
                    nc.tensor.matmul(dps[0:120, :],
                                     onest[0:120, 0:120], et[0:120, :],
                                     start=True, stop=True, tile_position=(0, 0))
                    dtmp = outp.tile([128, N_FREE], F32, tag=f"dtmp{unit}",
                                     name=f"dtmp{unit}")
                    nc.scalar.copy(dtmp[0:120, :], dps[0:120, :])
                    rr = outp.tile([128, N_FREE], FP16, tag=f"rr{unit}",
                                   name=f"rr{unit}")
                    rr32 = outp.tile([128, N_FREE], F32, tag=f"rr32{unit}",
                                     name=f"rr32{unit}")
                    nc.vector.reciprocal_approx_fast(rr32[0:120, :], dtmp[0:120, :])
                    nc.scalar.copy(rr[0:120, :], rr32[0:120, :])
                    ot = outp.tile([128, N_FREE], F32, tag=f"ot{unit}",
                                   name=f"ot{unit}")
                    nc.vector.tensor_tensor(
                        ot[0:120, :], et[0:120, :], rr[0:120, :],
                        mybir.AluOpType.mult)
                    nc.scalar.dma_start(y[h0:h0 + 5, :, b0:b0 + 8, :], ot[0:120, :])
    nc.finalize()
    return nc


def kernel(x, conv_weight, conv_bias):
    import ml_dtypes
    x = np.asarray(x, dtype=np.float32)
    conv_weight = np.asarray(conv_weight, dtype=np.float32)
    conv_bias = np.asarray(conv_bias, dtype=np.float32)
    L, ones, biasv = build_host_tensors(conv_weight, conv_bias)
    L = L.astype(ml_dtypes.bfloat16)
    ones = ones.astype(np.float16)
    if "nc" not in _CACHE:
        _CACHE["nc"] = build_bass()
    nc = _CACHE["nc"]
    core_ids = list(range(8))
    # (b, c, d, h, w) -> (d, h, c, b, w), cast to bf16 (halves DMA traffic)
    x_t = np.ascontiguousarray(
        np.transpose(x, (2, 3, 1, 0, 4)).astype(ml_dtypes.bfloat16))
    in_maps = []
    for i in core_ids:
        in_maps.append({
            "x": np.ascontiguousarray(x_t[:, :, :, i * B_CORE:(i + 1) * B_CORE, :]),
            "lw": L, "ones": ones, "biasv": biasv,
        })
    res = run_bass_kernel_spmd(nc, in_maps, core_ids)
    out = np.concatenate(
        [np.transpose(res.results[i]["y"], (2, 1, 0, 3)) for i in range(8)], axis=0)
    return np.ascontiguousarray(out)


if __name__ == "__main__":
    rng = np.random.default_rng(0)
    x = rng.standard_normal((128, 3, 16, 64, 64), dtype=np.float32)
    w = (rng.standard_normal((24, 3, 3, 3, 3)) * 0.1).astype(np.float32)
    b = (rng.standard_normal(24) * 0.1).astype(np.float32)
    out = kernel(x=x, conv_weight=w, conv_bias=b)
    print("out", out.shape, out.dtype)


# revision 33
# speedup vs baseline: 1.0004x; 1.0004x over previous
"""Trainium2 Bass kernel: Conv3d(3->24, k=3, valid) + bias -> min over depth -> softmax over channels.

Full inputs: x (128, 3, 16, 64, 64) f32, conv_weight (24, 3, 3, 3, 3), conv_bias (24,).
Output: (128, 24, 62, 62) f32.

Data-parallel over 8 cores (16 batch each). Per core:
  Conv as TensorE matmul, K = 63 partitions = 3 depth-taps x (7 h-rows x 3 in-chans),
  M = 128 = (5 h-outputs x 24 out-chans + 8 pad), N = 496 = (8 batch x 62 w-outputs).
  kw handled by 3 PSUM-accumulating passes with shifted rhs offsets; depth (14 outputs) sequential,
  each step loading a fresh pool-buffered 3-plane tile (deep DMA prefetch; ~3x input re-read is
  still under the PE roofline and removes load latency from the critical path).
  Two row-tiled PE units (SBUF partitions 0-62 / 64-126) process the two batch octets concurrently;
  slot loads are per-unit 3-dim DMAs with 2KB contiguous (b, w) runs (host pre-transposes x to
  (d, h, c, b, w)), issued on the SP HWDGE ring (unit 0) and gpsimd SWDGE (unit 1).
  Epilogue: min over depth split across ScalarE (psum->sbuf fp16 copies) and VectorE (running min),
  exp with per-partition bias, block-diag ones-matmul for softmax denominators, fast reciprocal,
  multiply; output written in (h, o, b, w) layout and transposed back on host.
"""

import numpy as np

import concourse.bacc as bacc
import concourse.bass as bass
import concourse.mybir as mybir
import concourse.tile as tile
from concourse.bass_utils import run_bass_kernel_spmd

F32 = mybir.dt.float32
F32R = mybir.dt.float32r
FP16 = mybir.dt.float16
BF16 = mybir.dt.bfloat16

B_CORE = 16
C_IN = 3
D_IN = 16
H_IN = 64
W_IN = 64
O = 24
D_OUT = 14
HW_OUT = 62
HB = [0, 5, 10, 15, 20, 25, 30, 35, 40, 45, 50, 55, 57]
N_FREE = 8 * HW_OUT  # 496

_CACHE = {}


def build_host_tensors(conv_weight, conv_bias):
    """lhsT variants (kw) with partition order p = kd*21 + j*3 + c."""
    L = np.zeros((3, 63, 128), np.float32)
    for kw in range(3):
        for kd in range(3):
            for c in range(C_IN):
                for j in range(7):
                    for hp in range(5):
                        kh = j - hp
                        if 0 <= kh <= 2:
                            L[kw, kd * 21 + j * 3 + c, hp * 24:hp * 24 + O] = \
                                conv_weight[:, c, kd, kh, kw]
    ones = np.zeros((128, 128), np.float32)
    for hp in range(5):
        ones[hp * 24:(hp + 1) * 24, hp * 24:(hp + 1) * 24] = 1.0
    biasv = np.zeros((128, 1), np.float32)
    for hp in range(5):
        biasv[hp * 24:(hp + 1) * 24, 0] = conv_bias
    Lp = np.ascontiguousarray(L.transpose(1, 0, 2).reshape(63, 3 * 128))
    return Lp, ones, biasv


def build_bass():
    nc = bacc.Bacc(None, target_bir_lowering=False)
    # x pre-transposed on host to (d, h, c, b, w): one slot DMA covers both units
    # with 3-dim APs and 2KB contiguous (b, w) runs.
    x = nc.dram_tensor("x", [D_IN, H_IN, C_IN, B_CORE, W_IN], F32R, kind="ExternalInput")
    lw = nc.dram_tensor("lw", [63, 3 * 128], F32R, kind="ExternalInput")
    ones = nc.dram_tensor("ones", [128, 128], FP16, kind="ExternalInput")
    biasv = nc.dram_tensor("biasv", [128, 1], F32, kind="ExternalInput")
    y = nc.dram_tensor("y", [HW_OUT, O, B_CORE, HW_OUT], F32, kind="ExternalOutput")

    with tile.TileContext(nc) as tc:
        with (
            tc.tile_pool(name="const", bufs=1) as constp,
            tc.tile_pool(name="xs", bufs=4) as xsp,
            tc.tile_pool(name="mins", bufs=2) as minsp,
            tc.tile_pool(name="acp", bufs=2) as acp,
            tc.tile_pool(name="evt", bufs=4) as evp,
            tc.tile_pool(name="outp", bufs=4) as outp,
            tc.tile_pool(name="ps", bufs=3, space="PSUM") as psp,
            tc.tile_pool(name="psd", bufs=1, space="PSUM") as psdp,
        ):
            lwt = constp.tile([128, 3 * 128], F32R, tag="lw")
            onest = constp.tile([128, 128], FP16, tag="ones")
            biast = constp.tile([128, 1], F32, tag="bias")
            for base in (0, 64):
                nc.sync.dma_start(lwt[base:base + 63, :], lw[:, :])
            nc.sync.dma_start(onest[:, :], ones[:, :])
            nc.sync.dma_start(biast[:, :], biasv[:, :])

            for h0 in HB:
                accs = [None, None]
                stash = [[], []]
                for dt in range(D_OUT):
                    xt = xsp.tile([128, 512], F32R, tag="xt", name="xt")
                    for unit, eng in ((0, nc.sync), (1, nc.gpsimd)):
                        base = 64 * unit
                        srcap = x[dt:dt + 3, h0:h0 + 7, :,
                                  unit * 8:unit * 8 + 8, :].rearrange(
                            "p j c b w -> p (j c) (b w)")
                        eng.dma_start(xt[base:base + 63, :], srcap)
                    for unit in range(2):
                        base = 64 * unit
                        ps = psp.tile([128, N_FREE], F32, tag=f"ps{unit}",
                                      name=f"ps{unit}")
                        for kw in range(3):
                            lhsT = lwt[base:base + 63,
                                       kw * 128:(kw + 1) * 128]
                            rhs = xt[base:base + 63, :].rearrange(
                                "p (b w) -> p b w", b=8)[:, :, kw:kw + HW_OUT]
                            nc.tensor.matmul(
                                ps[:, :], lhsT, rhs,
                                start=(kw == 0), stop=(kw == 2),
                                tile_position=(base, 0))
                        # min-chain: even dt (and 13) -> ScalarE copy (fp16), odd -> VectorE min
                        if dt % 2 == 0 or dt == 13:
                            a = acp.tile([128, N_FREE], FP16, tag=f"a{unit}",
                                         name=f"a{unit}", bufs=16)
                            nc.scalar.copy(a[:, :], ps[:, :])
                            stash[unit].append(a)
                        else:
                            acc = accs[unit]
                            if acc is None:
                                acc = minsp.tile([128, N_FREE], FP16,
                                                 tag=f"mins{unit}", name=f"mins{unit}")
                                nc.vector.tensor_tensor(
                                    acc[:, :], ps[:, :], stash[unit].pop(0)[:, :],
                                    mybir.AluOpType.min)
                                accs[unit] = acc
                            else:
                                nc.vector.tensor_tensor(
                                    acc[:, :], ps[:, :], acc[:, :],
                                    mybir.AluOpType.min)
                for unit in range(2):
                    b0 = unit * 8
                    acc = accs[unit]
                    for a in stash[unit]:
                        nc.vector.tensor_tensor(
                            acc[:, :], a[:, :], acc[:, :], mybir.AluOpType.min)
                    et = evp.tile([128, N_FREE], FP16, tag=f"e{unit}", name=f"et{unit}")
                    nc.scalar.activation(et[:, :], acc[:, :],
                                         mybir.ActivationFunctionType.Exp,
                                         bias=biast[:, 0:1], scale=1.0)
                    dps = psdp.tile([128, N_FREE], F32, tag=f"dps{unit}",
                                    name=f"dps{unit}")
                    nc.tensor.matmul(dps[0:120, :],
                                     onest[0:120, 0:120], et[0:120, :],
                                     start=True, stop=True, tile_position=(0, 0))
                    dtmp = outp.tile([128, N_FREE], F32, tag=f"dtmp{unit}",
                                     name=f"dtmp{unit}")
                    nc.scalar.copy(dtmp[0:120, :], dps[0:120, :])
                    rr = outp.tile([128, N_FREE], FP16, tag=f"rr{unit}",
                                   name=f"rr{unit}")
                    rr32 = outp.tile([128, N_FREE], F32, tag=f"rr32{unit}",
                                     name=f"rr32{unit}")
                    nc.vector.reciprocal_approx_fast(rr32[0:120, :], dtmp[0:120, :])
                    nc.scalar.copy(rr[0:120, :], rr32[0:120, :])
                    ot = outp.tile([128, N_FREE], F32, tag=f"ot{unit}",
                                   name=f"ot{unit}")
                    nc.vector.tensor_tensor(
                        ot[0:120, :], et[0:120, :], rr[0:120, :],
                        mybir.AluOpType.mult)
                    nc.scalar.dma_start(y[h0:h0 + 5, :, b0:b0 + 8, :], ot[0:120, :])
    nc.finalize()
    return nc


def kernel(x, conv_weight, conv_bias):
    import ml_dtypes
    x = np.asarray(x, dtype=np.float32)
    conv_weight = np.asarray(conv_weight, dtype=np.float32)
    conv_bias = np.asarray(conv_bias, dtype=np.float32)
    L, ones, biasv = build_host_tensors(conv_weight, conv_bias)
    ones = ones.astype(np.float16)
    if "nc" not in _CACHE:
        _CACHE["nc"] = build_bass()
    nc = _CACHE["nc"]
    core_ids = list(range(8))
    # (b, c, d, h, w) -> (d, h, c, b, w)
    x_t = np.ascontiguousarray(np.transpose(x, (2, 3, 1, 0, 4)))
    in_maps = []
    for i in core_ids:
        in_maps.append({
            "x": np.ascontiguousarray(x_t[:, :, :, i * B_CORE:(i + 1) * B_CORE, :]),
            "lw": L, "ones": ones, "biasv": biasv,
        })
    res = run_bass_kernel_spmd(nc, in_maps, core_ids)
    out = np.concatenate(
        [np.transpose(res.results[i]["y"], (2, 1, 0, 3)) for i in range(8)], axis=0)
    return np.ascontiguousarray(out)


if __name__ == "__main__":
    rng = np.random.default_rng(0)
    x = rng.standard_normal((128, 3, 16, 64, 64), dtype=np.float32)
    w = (rng.standard_normal((24, 3, 3, 3, 3)) * 0.1).astype(np.float32)
    b = (rng.standard_normal(24) * 0.1).astype(np.float32)
    out = kernel(x=x, conv_weight=w, conv_bias=b)
    print("out", out.shape, out.dtype)
